# revision 21
# baseline (speedup 1.0000x reference)
import numpy as np
import ml_dtypes
from contextlib import ExitStack

import concourse.bass as bass
import concourse.bacc as bacc
import concourse.tile as tile
from concourse import mybir
from concourse import bass_utils

K = 7
H = 16
B, S, D = 8, 2048, 1024
L = S - K + 1
C = D // 128
NSB = 4
SB = S // NSB
KH = K * H

F32 = mybir.dt.float32
F32R = mybir.dt.float32r
BF16 = mybir.dt.bfloat16

MM_DT = F32R


def _mm(ap):
    if MM_DT == F32:
        return ap
    return ap.bitcast(MM_DT)


def _host_constants():
    ident = np.eye(128, dtype=np.float32)
    identb = np.eye(128).astype(ml_dtypes.bfloat16)
    h = np.arange(KH) % H
    selsum = ((h[:, None] == h[None, :]) * float(K)).astype(ml_dtypes.bfloat16)
    selk = np.zeros((KH, K * 128), dtype=ml_dtypes.bfloat16)
    for k in range(K):
        for p in range(128):
            selk[16 * k + p % 16, k * 128 + p] = 1.0
    return ident, identb, selsum, selk


def build_program():
    nc = bacc.Bacc(
        "TRN2", target_bir_lowering=False, debug=False, enable_asserts=True
    )

    x_d = nc.dram_tensor("x", [S, D], F32, kind="ExternalInput").ap()
    w_d = nc.dram_tensor("W", [D, KH], BF16, kind="ExternalInput").ap()
    b_d = nc.dram_tensor("b", [KH], F32, kind="ExternalInput").ap()
    ident_d = nc.dram_tensor("ident", [128, 128], F32, kind="ExternalInput").ap()
    identb_d = nc.dram_tensor("identb", [128, 128], BF16, kind="ExternalInput").ap()
    selsum_d = nc.dram_tensor("selsum", [KH, KH], BF16, kind="ExternalInput").ap()
    selk_d = nc.dram_tensor("selk", [KH, K * 128], BF16, kind="ExternalInput").ap()
    out_d = nc.dram_tensor("out", [L, D], F32, kind="ExternalOutput").ap()

    with tile.TileContext(nc) as tc, ExitStack() as ctx:
        singles = ctx.enter_context(tc.tile_pool(name="singles", bufs=1))
        xn_pool = ctx.enter_context(tc.tile_pool(name="xn", bufs=2))
        prodv_pool = ctx.enter_context(tc.tile_pool(name="prodv", bufs=12))
        prodg_pool = ctx.enter_context(tc.tile_pool(name="prodg", bufs=8))
        outn_pool = ctx.enter_context(tc.tile_pool(name="outn", bufs=3))

        p_tp = ctx.enter_context(tc.tile_pool(name="ptp", bufs=2, space="PSUM"))
        p_log = ctx.enter_context(tc.tile_pool(name="plog", bufs=1, space="PSUM"))
        p_sum = ctx.enter_context(tc.tile_pool(name="psumk", bufs=1, space="PSUM"))
        p_mk = ctx.enter_context(tc.tile_pool(name="pmk", bufs=2, space="PSUM"))
        p_otp = ctx.enter_context(tc.tile_pool(name="potp", bufs=2, space="PSUM"))

        wt = singles.tile([128, C, KH], BF16)
        nc.sync.dma_start(out=wt, in_=w_d.rearrange("(c p) n -> p c n", p=128))
        bias_t = singles.tile([KH, 1], F32)
        nc.sync.dma_start(out=bias_t, in_=b_d.rearrange("(p one) -> p one", one=1))
        ident_t = singles.tile([128, 128], F32)
        nc.sync.dma_start(out=ident_t, in_=ident_d)
        identb_t = singles.tile([128, 128], BF16)
        nc.sync.dma_start(out=identb_t, in_=identb_d)
        selsum_t = singles.tile([KH, KH], BF16)
        nc.sync.dma_start(out=selsum_t, in_=selsum_d)
        selk_t = singles.tile([KH, K, 128], BF16)
        nc.sync.dma_start(
            out=selk_t, in_=selk_d.rearrange("c (k p) -> c k p", k=K)
        )

        xtb = singles.tile([128, C, S], BF16)
        e_full = singles.tile([KH, S], BF16)
        rinv = singles.tile([KH, S], F32)
        en = singles.tile([KH, S], BF16)
        m_all = singles.tile([128, K, S], BF16)
        acc_all = singles.tile([128, C, S], BF16)

        def front(sb):
            xn = xn_pool.tile([128, 4, D], F32, tag="xn")
            nc.sync.dma_start(
                out=xn,
                in_=x_d[SB * sb : SB * (sb + 1), :].rearrange(
                    "(t p) d -> p t d", p=128
                ),
            )
            for c in range(C):
                ptp = p_tp.tile([128, SB], F32, tag="ptp")
                for tt in range(4):
                    nc.tensor.transpose(
                        ptp[:, 128 * tt : 128 * (tt + 1)],
                        xn[:, tt, 128 * c : 128 * (c + 1)],
                        ident_t,
                    )
                nc.scalar.copy(xtb[:, c, SB * sb : SB * (sb + 1)], ptp)
            plog = p_log.tile([KH, SB], F32, tag="plog")
            for c in range(C):
                nc.tensor.matmul(
                    plog,
                    wt[:, c, :],
                    xtb[:, c, SB * sb : SB * (sb + 1)],
                    start=(c == 0),
                    stop=(c == C - 1),
                )
            nc.scalar.activation(
                e_full[:, SB * sb : SB * (sb + 1)],
                plog,
                mybir.ActivationFunctionType.Exp,
                bias=bias_t,
                scale=1.0,
            )

        def denom(sb):
            sl = slice(SB * sb, SB * (sb + 1))
            psum = p_sum.tile([KH, SB], F32, tag="psumk")
            nc.tensor.matmul(psum, selsum_t, e_full[:, sl], start=True, stop=True)
            nc.vector.reciprocal(rinv[:, sl], psum)
            nc.gpsimd.tensor_mul(en[:, sl], e_full[:, sl], rinv[:, sl])

        CB = [0, SB - K + 1, 2 * SB - K + 1, 3 * SB - K + 1, L]

        def mrep(j):
            l0, l1 = CB[j], CB[j + 1]
            nl = l1 - l0
            for k in range(K):
                pmk = p_mk.tile([128, SB], F32, tag="pmk")
                nc.tensor.matmul(
                    pmk[:, :nl],
                    selk_t[:, k, :],
                    en[:, l0 + K - 1 : l0 + K - 1 + nl],
                    start=True,
                    stop=True,
                )
                nc.scalar.copy(m_all[:, k, l0 : l0 + nl], pmk[:, :nl])

        def conv(c, j):
            l0, l1 = CB[j], CB[j + 1]
            nl = l1 - l0

            def prod(eng, k, pool, tag):
                p = pool.tile([128, SB], BF16, tag=tag)
                eng.tensor_mul(
                    p[:, :nl],
                    m_all[:, k, l0 : l0 + nl],
                    xtb[:, c, l0 + k : l0 + k + nl],
                )
                return p

            p5 = prod(nc.gpsimd, 5, prodg_pool, "prodg")
            p6 = prod(nc.gpsimd, 6, prodg_pool, "prodg")
            a56 = prodg_pool.tile([128, SB], BF16, tag="prodg")
            nc.gpsimd.tensor_add(a56[:, :nl], p5[:, :nl], p6[:, :nl])
            p0 = prod(nc.vector, 0, prodv_pool, "prodv")
            p1 = prod(nc.vector, 1, prodv_pool, "prodv")
            a01 = prodv_pool.tile([128, SB], BF16, tag="prodv")
            nc.vector.tensor_add(a01[:, :nl], p0[:, :nl], p1[:, :nl])
            p2 = prod(nc.vector, 2, prodv_pool, "prodv")
            p3 = prod(nc.vector, 3, prodv_pool, "prodv")
            a23 = prodv_pool.tile([128, SB], BF16, tag="prodv")
            nc.vector.tensor_add(a23[:, :nl], p2[:, :nl], p3[:, :nl])
            p4 = prod(nc.vector, 4, prodv_pool, "prodv")
            t0 = prodv_pool.tile([128, SB], BF16, tag="prodv")
            nc.vector.tensor_add(t0[:, :nl], a01[:, :nl], a23[:, :nl])
            t1 = prodv_pool.tile([128, SB], BF16, tag="prodv")
            nc.vector.tensor_add(t1[:, :nl], p4[:, :nl], a56[:, :nl])
            nc.vector.tensor_add(
                acc_all[:, c, l0 : l0 + nl], t0[:, :nl], t1[:, :nl]
            )

        def store(lb):
            l0 = 128 * lb
            nl = min(128, L - l0)
            outn = outn_pool.tile([128, D], F32, tag="outn")
            for half in range(2):
                potp = p_otp.tile([128, 512], BF16, tag="potp")
                for cc in range(4):
                    c = 4 * half + cc
                    nc.tensor.transpose(
                        potp[:nl, 128 * cc : 128 * (cc + 1)],
                        acc_all[:, c, l0 : l0 + nl],
                        identb_t,
                    )
                nc.scalar.copy(outn[:nl, 512 * half : 512 * (half + 1)], potp[:nl, :])
            nc.sync.dma_start(out=out_d[l0 : l0 + nl, :], in_=outn[:nl, :])

        STORE_OF_CHUNK = {0: range(0, 3), 1: range(3, 7), 2: range(7, 11), 3: range(11, 16)}
        front(0)
        denom(0)
        mrep(0)
        front(1)
        denom(1)
        mrep(1)
        for c in range(C):
            conv(c, 0)
        for lb in STORE_OF_CHUNK[0]:
            store(lb)
        front(2)
        denom(2)
        mrep(2)
        for c in range(C):
            conv(c, 1)
        for lb in STORE_OF_CHUNK[1]:
            store(lb)
        front(3)
        denom(3)
        mrep(3)
        for c in range(C):
            conv(c, 2)
        for lb in STORE_OF_CHUNK[2]:
            store(lb)
        for c in range(C):
            conv(c, 3)
        for lb in STORE_OF_CHUNK[3]:
            store(lb)

    nc.compile()
    return nc


_CACHE = {}


def _get_program():
    if "nc" not in _CACHE:
        _CACHE["nc"] = build_program()
    return _CACHE["nc"]


def kernel(x, W, b):
    x = np.asarray(x, dtype=np.float32)
    W = np.asarray(W, dtype=np.float32).astype(ml_dtypes.bfloat16)
    b = np.asarray(b, dtype=np.float32)
    assert x.shape == (B, S, D), x.shape

    nc = _get_program()
    ident, identb, selsum, selk = _host_constants()
    in_maps = []
    for core in range(B):
        in_maps.append(
            {
                "x": np.ascontiguousarray(x[core]),
                "W": W,
                "b": b,
                "ident": ident,
                "identb": identb,
                "selsum": selsum,
                "selk": selk,
            }
        )
    res = bass_utils.run_bass_kernel_spmd(nc, in_maps, core_ids=list(range(B)))
    out = np.stack([res.results[core]["out"] for core in range(B)], axis=0)
    return out


# revision 29
# speedup vs baseline: 1.0390x; 1.0390x over previous
import numpy as np
import ml_dtypes
from contextlib import ExitStack

import concourse.bass as bass
import concourse.bacc as bacc
import concourse.tile as tile
from concourse import mybir
from concourse import bass_utils

K = 7
H = 16
B, S, D = 8, 2048, 1024
L = S - K + 1
C = D // 128
NSB = 4
SB = S // NSB
KH = K * H

F32 = mybir.dt.float32
F32R = mybir.dt.float32r
BF16 = mybir.dt.bfloat16

MM_DT = F32R


def _mm(ap):
    if MM_DT == F32:
        return ap
    return ap.bitcast(MM_DT)


def _host_constants():
    ident = np.eye(128, dtype=np.float32)
    identb = np.eye(128).astype(ml_dtypes.bfloat16)
    h = np.arange(KH) % H
    selsum = ((h[:, None] == h[None, :]) * float(K)).astype(ml_dtypes.bfloat16)
    selk = np.zeros((KH, K * 128), dtype=ml_dtypes.bfloat16)
    for k in range(K):
        for p in range(128):
            selk[16 * k + p % 16, k * 128 + p] = 1.0
    return ident, identb, selsum, selk


def build_program():
    nc = bacc.Bacc(
        "TRN2", target_bir_lowering=False, debug=False, enable_asserts=True
    )

    x_d = nc.dram_tensor("x", [S, D], F32, kind="ExternalInput").ap()
    w_d = nc.dram_tensor("W", [D, KH], BF16, kind="ExternalInput").ap()
    b_d = nc.dram_tensor("b", [KH], F32, kind="ExternalInput").ap()
    ident_d = nc.dram_tensor("ident", [128, 128], F32, kind="ExternalInput").ap()
    identb_d = nc.dram_tensor("identb", [128, 128], BF16, kind="ExternalInput").ap()
    selsum_d = nc.dram_tensor("selsum", [KH, KH], BF16, kind="ExternalInput").ap()
    selk_d = nc.dram_tensor("selk", [KH, K * 128], BF16, kind="ExternalInput").ap()
    out_d = nc.dram_tensor("out", [L, D], F32, kind="ExternalOutput").ap()

    with tile.TileContext(nc) as tc, ExitStack() as ctx:
        singles = ctx.enter_context(tc.tile_pool(name="singles", bufs=1))
        xn_pool = ctx.enter_context(tc.tile_pool(name="xn", bufs=3))
        prodv_pool = ctx.enter_context(tc.tile_pool(name="prodv", bufs=8))
        prodg_pool = ctx.enter_context(tc.tile_pool(name="prodg", bufs=6))
        outn_pool = ctx.enter_context(tc.tile_pool(name="outn", bufs=3))

        p_tp = ctx.enter_context(tc.tile_pool(name="ptp", bufs=2, space="PSUM"))
        p_log = ctx.enter_context(tc.tile_pool(name="plog", bufs=1, space="PSUM"))
        p_sum = ctx.enter_context(tc.tile_pool(name="psumk", bufs=1, space="PSUM"))
        p_mk = ctx.enter_context(tc.tile_pool(name="pmk", bufs=2, space="PSUM"))
        p_otp = ctx.enter_context(tc.tile_pool(name="potp", bufs=2, space="PSUM"))

        wt = singles.tile([128, C, KH], BF16)
        nc.sync.dma_start(out=wt, in_=w_d.rearrange("(c p) n -> p c n", p=128))
        bias_t = singles.tile([KH, 1], F32)
        nc.sync.dma_start(out=bias_t, in_=b_d.rearrange("(p one) -> p one", one=1))
        ident_t = singles.tile([128, 128], F32)
        nc.sync.dma_start(out=ident_t, in_=ident_d)
        identb_t = singles.tile([128, 128], BF16)
        nc.sync.dma_start(out=identb_t, in_=identb_d)
        selsum_t = singles.tile([KH, KH], BF16)
        nc.sync.dma_start(out=selsum_t, in_=selsum_d)
        selk_t = singles.tile([KH, K, 128], BF16)
        nc.sync.dma_start(
            out=selk_t, in_=selk_d.rearrange("c (k p) -> c k p", k=K)
        )

        warm = singles.tile([1, 8], BF16)
        nc.gpsimd.tensor_mul(warm, identb_t[:1, :8], identb_t[:1, :8])

        xtb = singles.tile([128, C, S], BF16)
        e_full = singles.tile([KH, S], BF16)
        rinv = singles.tile([KH, S], F32)
        en = singles.tile([KH, S], BF16)
        acc_all = singles.tile([128, C, S], BF16)

        xn_tiles = {}

        def load(sb):
            xn = xn_pool.tile([128, 4, D], F32, tag="xn")
            nc.sync.dma_start(
                out=xn,
                in_=x_d[SB * sb : SB * (sb + 1), :].rearrange(
                    "(t p) d -> p t d", p=128
                ),
            )
            xn_tiles[sb] = xn

        def front(sb):
            xn = xn_tiles[sb]
            for c in range(C):
                ptp = p_tp.tile([128, SB], F32, tag="ptp")
                for tt in range(4):
                    nc.tensor.transpose(
                        ptp[:, 128 * tt : 128 * (tt + 1)],
                        xn[:, tt, 128 * c : 128 * (c + 1)],
                        ident_t,
                    )
                nc.scalar.copy(xtb[:, c, SB * sb : SB * (sb + 1)], ptp)
            plog = p_log.tile([KH, SB], F32, tag="plog")
            for c in range(C):
                nc.tensor.matmul(
                    plog,
                    wt[:, c, :],
                    xtb[:, c, SB * sb : SB * (sb + 1)],
                    start=(c == 0),
                    stop=(c == C - 1),
                )
            nc.scalar.activation(
                e_full[:, SB * sb : SB * (sb + 1)],
                plog,
                mybir.ActivationFunctionType.Exp,
                bias=bias_t,
                scale=1.0,
            )

        def denom(sb):
            sl = slice(SB * sb, SB * (sb + 1))
            psum = p_sum.tile([KH, SB], F32, tag="psumk")
            nc.tensor.matmul(psum, selsum_t, e_full[:, sl], start=True, stop=True)
            nc.vector.reciprocal(rinv[:, sl], psum)
            nc.gpsimd.tensor_mul(en[:, sl], e_full[:, sl], rinv[:, sl])

        m_pool = ctx.enter_context(tc.tile_pool(name="mw", bufs=2))
        m_tiles = {}

        CB = [0, SB - K + 1, 2 * SB - K + 1, 3 * SB - K + 1, L]
        CH = [0, 2 * SB - K + 1, L]

        def mrep(j):
            h, off = (j // 2), CB[j] - CH[j // 2]
            if j % 2 == 0:
                mt_new = m_pool.tile([128, K, 2 * SB], BF16, tag="mw")
                m_tiles[h] = mt_new
            mt = m_tiles[h]
            l0, l1 = CB[j], CB[j + 1]
            nl = l1 - l0
            for k in range(K):
                pmk = p_mk.tile([128, SB], F32, tag="pmk")
                nc.tensor.matmul(
                    pmk[:, :nl],
                    selk_t[:, k, :],
                    en[:, l0 + K - 1 : l0 + K - 1 + nl],
                    start=True,
                    stop=True,
                )
                nc.scalar.copy(mt[:, k, off : off + nl], pmk[:, :nl])

        def conv(c, h):
            l0, l1 = CH[h], CH[h + 1]
            nl = l1 - l0

            def prod(eng, k, pool, tag):
                p = pool.tile([128, 2 * SB], BF16, tag=tag)
                eng.tensor_mul(
                    p[:, :nl],
                    m_tiles[h][:, k, :nl],
                    xtb[:, c, l0 + k : l0 + k + nl],
                )
                return p

            p5 = prod(nc.gpsimd, 5, prodg_pool, "prodg")
            p6 = prod(nc.gpsimd, 6, prodg_pool, "prodg")
            a56 = prodg_pool.tile([128, 2 * SB], BF16, tag="prodg")
            nc.gpsimd.tensor_add(a56[:, :nl], p5[:, :nl], p6[:, :nl])
            p0 = prod(nc.vector, 0, prodv_pool, "prodv")
            p1 = prod(nc.vector, 1, prodv_pool, "prodv")
            a01 = prodv_pool.tile([128, 2 * SB], BF16, tag="prodv")
            nc.vector.tensor_add(a01[:, :nl], p0[:, :nl], p1[:, :nl])
            p2 = prod(nc.vector, 2, prodv_pool, "prodv")
            p3 = prod(nc.vector, 3, prodv_pool, "prodv")
            a23 = prodv_pool.tile([128, 2 * SB], BF16, tag="prodv")
            nc.vector.tensor_add(a23[:, :nl], p2[:, :nl], p3[:, :nl])
            p4 = prod(nc.vector, 4, prodv_pool, "prodv")
            t0 = prodv_pool.tile([128, 2 * SB], BF16, tag="prodv")
            nc.vector.tensor_add(t0[:, :nl], a01[:, :nl], a23[:, :nl])
            t1 = prodv_pool.tile([128, 2 * SB], BF16, tag="prodv")
            nc.vector.tensor_add(t1[:, :nl], p4[:, :nl], a56[:, :nl])
            nc.vector.tensor_add(
                acc_all[:, c, l0 : l0 + nl], t0[:, :nl], t1[:, :nl]
            )

        def store(lb):
            l0 = 128 * lb
            nl = min(128, L - l0)
            outn = outn_pool.tile([128, D], F32, tag="outn")
            for half in range(2):
                potp = p_otp.tile([128, 512], BF16, tag="potp")
                for cc in range(4):
                    c = 4 * half + cc
                    nc.tensor.transpose(
                        potp[:nl, 128 * cc : 128 * (cc + 1)],
                        acc_all[:, c, l0 : l0 + nl],
                        identb_t,
                    )
                nc.scalar.copy(outn[:nl, 512 * half : 512 * (half + 1)], potp[:nl, :])
            nc.scalar.dma_start(out=out_d[l0 : l0 + nl, :], in_=outn[:nl, :])

        STORE_OF_HALF = {0: range(0, 7), 1: range(7, 16)}
        for j in range(4):
            load(j)
        for j in range(4):
            front(j)
            denom(j)
            mrep(j)
            if j == 1:
                for c in range(C):
                    conv(c, 0)
                for lb in STORE_OF_HALF[0]:
                    store(lb)
        for c in range(C):
            conv(c, 1)
        for lb in STORE_OF_HALF[1]:
            store(lb)

    nc.compile()
    return nc


_CACHE = {}


def _get_program():
    if "nc" not in _CACHE:
        _CACHE["nc"] = build_program()
    return _CACHE["nc"]


def kernel(x, W, b):
    x = np.asarray(x, dtype=np.float32)
    W = np.asarray(W, dtype=np.float32).astype(ml_dtypes.bfloat16)
    b = np.asarray(b, dtype=np.float32)
    assert x.shape == (B, S, D), x.shape

    nc = _get_program()
    ident, identb, selsum, selk = _host_constants()
    in_maps = []
    for core in range(B):
        in_maps.append(
            {
                "x": np.ascontiguousarray(x[core]),
                "W": W,
                "b": b,
                "ident": ident,
                "identb": identb,
                "selsum": selsum,
                "selk": selk,
            }
        )
    res = bass_utils.run_bass_kernel_spmd(nc, in_maps, core_ids=list(range(B)))
    out = np.stack([res.results[core]["out"] for core in range(B)], axis=0)
    return out


# revision 35
# speedup vs baseline: 1.1200x; 1.0779x over previous
import numpy as np
import ml_dtypes
from contextlib import ExitStack

import concourse.bass as bass
import concourse.bacc as bacc
import concourse.tile as tile
from concourse.tile_rust import add_dep_helper
from concourse import mybir
from concourse import bass_utils

K = 7
H = 16
B, S, D = 8, 2048, 1024
L = S - K + 1
C = D // 128
NSB = 4
SB = S // NSB
KH = K * H

F32 = mybir.dt.float32
F32R = mybir.dt.float32r
BF16 = mybir.dt.bfloat16

MM_DT = F32R


def _mm(ap):
    if MM_DT == F32:
        return ap
    return ap.bitcast(MM_DT)


def _host_constants():
    ident = np.eye(128, dtype=np.float32)
    identb = np.eye(128).astype(ml_dtypes.bfloat16)
    h = np.arange(KH) % H
    selsum = ((h[:, None] == h[None, :]) * float(K)).astype(ml_dtypes.bfloat16)
    selk = np.zeros((KH, K * 128), dtype=ml_dtypes.bfloat16)
    for k in range(K):
        for p in range(128):
            selk[16 * k + p % 16, k * 128 + p] = 1.0
    return ident, identb, selsum, selk


def build_program():
    nc = bacc.Bacc(
        "TRN2", target_bir_lowering=False, debug=False, enable_asserts=True
    )

    x_d = nc.dram_tensor("x", [S, D], F32, kind="ExternalInput").ap()
    w_d = nc.dram_tensor("W", [D, KH], BF16, kind="ExternalInput").ap()
    b_d = nc.dram_tensor("b", [KH], F32, kind="ExternalInput").ap()
    ident_d = nc.dram_tensor("ident", [128, 128], F32, kind="ExternalInput").ap()
    identb_d = nc.dram_tensor("identb", [128, 128], BF16, kind="ExternalInput").ap()
    selsum_d = nc.dram_tensor("selsum", [KH, KH], BF16, kind="ExternalInput").ap()
    selk_d = nc.dram_tensor("selk", [KH, K * 128], BF16, kind="ExternalInput").ap()
    out_d = nc.dram_tensor("out", [L, D], F32, kind="ExternalOutput").ap()

    with tile.TileContext(nc) as tc, ExitStack() as ctx:
        singles = ctx.enter_context(tc.tile_pool(name="singles", bufs=1))
        xn_pool = ctx.enter_context(tc.tile_pool(name="xn", bufs=3))
        prodv_pool = ctx.enter_context(tc.tile_pool(name="prodv", bufs=8))
        prodg_pool = ctx.enter_context(tc.tile_pool(name="prodg", bufs=6))
        outn_pool = ctx.enter_context(tc.tile_pool(name="outn", bufs=3))

        p_tp = ctx.enter_context(tc.tile_pool(name="ptp", bufs=2, space="PSUM"))
        p_log = ctx.enter_context(tc.tile_pool(name="plog", bufs=1, space="PSUM"))
        p_sum = ctx.enter_context(tc.tile_pool(name="psumk", bufs=1, space="PSUM"))
        p_mk = ctx.enter_context(tc.tile_pool(name="pmk", bufs=2, space="PSUM"))
        p_otp = ctx.enter_context(tc.tile_pool(name="potp", bufs=2, space="PSUM"))

        wt = singles.tile([128, C, KH], BF16)
        nc.sync.dma_start(out=wt, in_=w_d.rearrange("(c p) n -> p c n", p=128))
        bias_t = singles.tile([KH, 1], F32)
        nc.sync.dma_start(out=bias_t, in_=b_d.rearrange("(p one) -> p one", one=1))
        ident_t = singles.tile([128, 128], F32)
        nc.sync.dma_start(out=ident_t, in_=ident_d)
        identb_t = singles.tile([128, 128], BF16)
        nc.sync.dma_start(out=identb_t, in_=identb_d)
        selsum_t = singles.tile([KH, KH], BF16)
        nc.sync.dma_start(out=selsum_t, in_=selsum_d)
        selk_t = singles.tile([KH, K, 128], BF16)
        nc.sync.dma_start(
            out=selk_t, in_=selk_d.rearrange("c (k p) -> c k p", k=K)
        )

        warm = singles.tile([1, 8], BF16)
        nc.gpsimd.tensor_mul(warm, identb_t[:1, :8], identb_t[:1, :8])

        xtb = singles.tile([128, C, S], BF16)
        e_full = singles.tile([KH, S], BF16)
        rinv = singles.tile([KH, S], F32)
        en = singles.tile([KH, S], BF16)
        acc_all = singles.tile([128, C, S], BF16)

        xn_tiles = {}

        def load(sb):
            xn = xn_pool.tile([128, 4, D], F32, tag="xn")
            nc.sync.dma_start(
                out=xn,
                in_=x_d[SB * sb : SB * (sb + 1), :].rearrange(
                    "(t p) d -> p t d", p=128
                ),
            )
            xn_tiles[sb] = xn

        def front(sb, hold=None):
            xn = xn_tiles[sb]
            for c in range(C):
                ptp = p_tp.tile([128, SB], F32, tag="ptp")
                for tt in range(4):
                    tp = nc.tensor.transpose(
                        ptp[:, 128 * tt : 128 * (tt + 1)],
                        xn[:, tt, 128 * c : 128 * (c + 1)],
                        ident_t,
                    )
                    if hold is not None:
                        add_dep_helper(tp.ins, hold.ins, sync=False,
                                       reason="pe order: front after prev sums")
                nc.scalar.copy(xtb[:, c, SB * sb : SB * (sb + 1)], ptp)
            plog = p_log.tile([KH, SB], F32, tag="plog")
            for c in range(C):
                nc.tensor.matmul(
                    plog,
                    wt[:, c, :],
                    xtb[:, c, SB * sb : SB * (sb + 1)],
                    start=(c == 0),
                    stop=(c == C - 1),
                )
            nc.scalar.activation(
                e_full[:, SB * sb : SB * (sb + 1)],
                plog,
                mybir.ActivationFunctionType.Exp,
                bias=bias_t,
                scale=1.0,
            )

        def denom(sb):
            sl = slice(SB * sb, SB * (sb + 1))
            psum = p_sum.tile([KH, SB], F32, tag="psumk")
            mm = nc.tensor.matmul(psum, selsum_t, e_full[:, sl], start=True, stop=True)
            nc.vector.reciprocal(rinv[:, sl], psum)
            nc.vector.tensor_mul(en[:, sl], e_full[:, sl], rinv[:, sl])
            return mm

        m_pool = ctx.enter_context(tc.tile_pool(name="mw", bufs=2))
        m_tiles = {}

        CB = [0, SB - K + 1, 2 * SB - K + 1, 3 * SB - K + 1, L]
        CH = [0, 2 * SB - K + 1, L]

        def mrep(j):
            h, off = (j // 2), CB[j] - CH[j // 2]
            if j % 2 == 0:
                mt_new = m_pool.tile([128, K, 2 * SB], BF16, tag="mw")
                m_tiles[h] = mt_new
            mt = m_tiles[h]
            l0, l1 = CB[j], CB[j + 1]
            nl = l1 - l0
            for k in range(K):
                pmk = p_mk.tile([128, SB], F32, tag="pmk")
                nc.tensor.matmul(
                    pmk[:, :nl],
                    selk_t[:, k, :],
                    en[:, l0 + K - 1 : l0 + K - 1 + nl],
                    start=True,
                    stop=True,
                )
                nc.scalar.copy(mt[:, k, off : off + nl], pmk[:, :nl])

        def conv(c, h, l0, l1, gp3=True):
            nl = l1 - l0
            off = l0 - CH[h]

            def prod(eng, k, pool, tag):
                p = pool.tile([128, 2 * SB], BF16, tag=tag)
                eng.tensor_mul(
                    p[:, :nl],
                    m_tiles[h][:, k, off : off + nl],
                    xtb[:, c, l0 + k : l0 + k + nl],
                )
                return p

            p5 = prod(nc.gpsimd, 5, prodg_pool, "prodg")
            p6 = prod(nc.gpsimd, 6, prodg_pool, "prodg")
            a56 = prodg_pool.tile([128, 2 * SB], BF16, tag="prodg")
            (nc.gpsimd if gp3 else nc.vector).tensor_add(
                a56[:, :nl], p5[:, :nl], p6[:, :nl]
            )
            p0 = prod(nc.vector, 0, prodv_pool, "prodv")
            p1 = prod(nc.vector, 1, prodv_pool, "prodv")
            a01 = prodv_pool.tile([128, 2 * SB], BF16, tag="prodv")
            nc.vector.tensor_add(a01[:, :nl], p0[:, :nl], p1[:, :nl])
            p2 = prod(nc.vector, 2, prodv_pool, "prodv")
            p3 = prod(nc.vector, 3, prodv_pool, "prodv")
            a23 = prodv_pool.tile([128, 2 * SB], BF16, tag="prodv")
            nc.vector.tensor_add(a23[:, :nl], p2[:, :nl], p3[:, :nl])
            p4 = prod(nc.vector, 4, prodv_pool, "prodv")
            t0 = prodv_pool.tile([128, 2 * SB], BF16, tag="prodv")
            nc.vector.tensor_add(t0[:, :nl], a01[:, :nl], a23[:, :nl])
            t1 = prodv_pool.tile([128, 2 * SB], BF16, tag="prodv")
            nc.vector.tensor_add(t1[:, :nl], p4[:, :nl], a56[:, :nl])
            nc.vector.tensor_add(
                acc_all[:, c, l0 : l0 + nl], t0[:, :nl], t1[:, :nl]
            )

        def store(lb):
            l0 = 128 * lb
            nl = min(128, L - l0)
            outn = outn_pool.tile([128, D], F32, tag="outn")
            for half in range(2):
                potp = p_otp.tile([128, 512], BF16, tag="potp")
                for cc in range(4):
                    c = 4 * half + cc
                    nc.tensor.transpose(
                        potp[:nl, 128 * cc : 128 * (cc + 1)],
                        acc_all[:, c, l0 : l0 + nl],
                        identb_t,
                    )
                nc.scalar.copy(outn[:nl, 512 * half : 512 * (half + 1)], potp[:nl, :])
            nc.scalar.dma_start(out=out_d[l0 : l0 + nl, :], in_=outn[:nl, :])

        for j in range(4):
            load(j)
        prev_sums = None
        front(0)
        front(1)
        prev_sums = denom(0)
        mrep(0)
        prev_sums = denom(1)
        mrep(1)
        for c in range(C):
            conv(c, 0, CB[0], CB[1])
        for lb in range(0, 3):
            store(lb)
        front(2, hold=prev_sums)
        prev_sums = denom(2)
        mrep(2)
        for c in range(C):
            conv(c, 0, CB[1], CB[2])
        for lb in range(3, 7):
            store(lb)
        front(3, hold=prev_sums)
        denom(3)
        mrep(3)
        for c in range(C):
            conv(c, 1, CB[2], CB[3])
        for lb in range(7, 11):
            store(lb)
        for c in range(C):
            conv(c, 1, CB[3], CH[2])
        for lb in range(11, 16):
            store(lb)

    nc.compile()
    return nc


_CACHE = {}


def _get_program():
    if "nc" not in _CACHE:
        _CACHE["nc"] = build_program()
    return _CACHE["nc"]


def kernel(x, W, b):
    x = np.asarray(x, dtype=np.float32)
    W = np.asarray(W, dtype=np.float32).astype(ml_dtypes.bfloat16)
    b = np.asarray(b, dtype=np.float32)
    assert x.shape == (B, S, D), x.shape

    nc = _get_program()
    ident, identb, selsum, selk = _host_constants()
    in_maps = []
    for core in range(B):
        in_maps.append(
            {
                "x": np.ascontiguousarray(x[core]),
                "W": W,
                "b": b,
                "ident": ident,
                "identb": identb,
                "selsum": selsum,
                "selk": selk,
            }
        )
    res = bass_utils.run_bass_kernel_spmd(nc, in_maps, core_ids=list(range(B)))
    out = np.stack([res.results[core]["out"] for core in range(B)], axis=0)
    return out


# revision 36
# speedup vs baseline: 1.1367x; 1.0150x over previous
import numpy as np
import ml_dtypes
from contextlib import ExitStack

import concourse.bass as bass
import concourse.bacc as bacc
import concourse.tile as tile
from concourse.tile_rust import add_dep_helper
from concourse import mybir
from concourse import bass_utils

K = 7
H = 16
B, S, D = 8, 2048, 1024
L = S - K + 1
C = D // 128
NSB = 4
SB = S // NSB
KH = K * H

F32 = mybir.dt.float32
F32R = mybir.dt.float32r
BF16 = mybir.dt.bfloat16

MM_DT = F32R


def _mm(ap):
    if MM_DT == F32:
        return ap
    return ap.bitcast(MM_DT)


_OFF_BIAS = 0
_OFF_IDENT = 4
_OFF_IDENTB = 516
_OFF_SELSUM = 772
_OFF_SELK = 996
_OFF_WT = 2788
_CONST_BYTES = 4580


def _host_constants(W, b):
    buf = np.zeros((128, _CONST_BYTES), np.uint8)

    def put(off, arr):
        by = np.ascontiguousarray(arr).view(np.uint8).reshape(arr.shape[0], -1)
        buf[: arr.shape[0], off : off + by.shape[1]] = by

    put(_OFF_BIAS, np.asarray(b, np.float32).reshape(KH, 1))
    put(_OFF_IDENT, np.eye(128, dtype=np.float32))
    put(_OFF_IDENTB, np.eye(128).astype(ml_dtypes.bfloat16))
    h = np.arange(KH) % H
    selsum = ((h[:, None] == h[None, :]) * float(K)).astype(ml_dtypes.bfloat16)
    put(_OFF_SELSUM, selsum)
    selk = np.zeros((KH, K * 128), dtype=ml_dtypes.bfloat16)
    for k in range(K):
        for p in range(128):
            selk[16 * k + p % 16, k * 128 + p] = 1.0
    put(_OFF_SELK, selk)
    wt = np.asarray(W, np.float32).astype(ml_dtypes.bfloat16)
    wt = wt.reshape(C, 128, KH).transpose(1, 0, 2).reshape(128, C * KH)
    put(_OFF_WT, np.ascontiguousarray(wt))
    return buf.view(np.float32)


def build_program():
    nc = bacc.Bacc(
        "TRN2", target_bir_lowering=False, debug=False, enable_asserts=True
    )

    x_d = nc.dram_tensor("x", [S, D], F32, kind="ExternalInput").ap()
    consts_d = nc.dram_tensor(
        "consts", [128, _CONST_BYTES // 4], F32, kind="ExternalInput"
    ).ap()
    out_d = nc.dram_tensor("out", [L, D], F32, kind="ExternalOutput").ap()

    with tile.TileContext(nc) as tc, ExitStack() as ctx:
        singles = ctx.enter_context(tc.tile_pool(name="singles", bufs=1))
        xn_pool = ctx.enter_context(tc.tile_pool(name="xn", bufs=3))
        prodv_pool = ctx.enter_context(tc.tile_pool(name="prodv", bufs=8))
        prodg_pool = ctx.enter_context(tc.tile_pool(name="prodg", bufs=6))
        outn_pool = ctx.enter_context(tc.tile_pool(name="outn", bufs=3))

        p_tp = ctx.enter_context(tc.tile_pool(name="ptp", bufs=2, space="PSUM"))
        p_log = ctx.enter_context(tc.tile_pool(name="plog", bufs=1, space="PSUM"))
        p_sum = ctx.enter_context(tc.tile_pool(name="psumk", bufs=1, space="PSUM"))
        p_mk = ctx.enter_context(tc.tile_pool(name="pmk", bufs=2, space="PSUM"))
        p_otp = ctx.enter_context(tc.tile_pool(name="potp", bufs=2, space="PSUM"))

        cblob = singles.tile([128, _CONST_BYTES // 4], F32)
        nc.sync.dma_start(out=cblob, in_=consts_d)
        cbytes = cblob.bitcast(mybir.dt.uint8)

        def cview(off, nbytes, dt, rows=128):
            return cbytes[:rows, off : off + nbytes].bitcast(dt)

        bias_t = cview(_OFF_BIAS, 4, F32, rows=KH)
        ident_t = cview(_OFF_IDENT, 512, F32)
        identb_t = cview(_OFF_IDENTB, 256, BF16)
        selsum_t = cview(_OFF_SELSUM, 224, BF16, rows=KH)
        selk_t = cview(_OFF_SELK, 1792, BF16, rows=KH).rearrange(
            "c (k p) -> c k p", k=K
        )
        wt = cview(_OFF_WT, 1792, BF16).rearrange("p (c n) -> p c n", c=C)

        warm = singles.tile([1, 8], BF16)
        nc.gpsimd.tensor_mul(warm, identb_t[:1, :8], identb_t[:1, :8])

        xtb = singles.tile([128, C, S], BF16)
        e_full = singles.tile([KH, S], BF16)
        rinv = singles.tile([KH, S], F32)
        en = singles.tile([KH, S], BF16)
        acc_all = singles.tile([128, C, S], BF16)

        xn_tiles = {}

        def load(sb):
            xn = xn_pool.tile([128, 4, D], F32, tag="xn")
            nc.sync.dma_start(
                out=xn,
                in_=x_d[SB * sb : SB * (sb + 1), :].rearrange(
                    "(t p) d -> p t d", p=128
                ),
            )
            xn_tiles[sb] = xn

        def front(sb, hold=None):
            xn = xn_tiles[sb]
            for c in range(C):
                ptp = p_tp.tile([128, SB], F32, tag="ptp")
                for tt in range(4):
                    tp = nc.tensor.transpose(
                        ptp[:, 128 * tt : 128 * (tt + 1)],
                        xn[:, tt, 128 * c : 128 * (c + 1)],
                        ident_t,
                    )
                    if hold is not None:
                        add_dep_helper(tp.ins, hold.ins, sync=False,
                                       reason="pe order: front after prev sums")
                nc.scalar.copy(xtb[:, c, SB * sb : SB * (sb + 1)], ptp)
            plog = p_log.tile([KH, SB], F32, tag="plog")
            for c in range(C):
                nc.tensor.matmul(
                    plog,
                    wt[:, c, :],
                    xtb[:, c, SB * sb : SB * (sb + 1)],
                    start=(c == 0),
                    stop=(c == C - 1),
                )
            nc.scalar.activation(
                e_full[:, SB * sb : SB * (sb + 1)],
                plog,
                mybir.ActivationFunctionType.Exp,
                bias=bias_t,
                scale=1.0,
            )

        def denom(sb):
            sl = slice(SB * sb, SB * (sb + 1))
            psum = p_sum.tile([KH, SB], F32, tag="psumk")
            mm = nc.tensor.matmul(psum, selsum_t, e_full[:, sl], start=True, stop=True)
            nc.vector.reciprocal(rinv[:, sl], psum)
            nc.vector.tensor_mul(en[:, sl], e_full[:, sl], rinv[:, sl])
            return mm

        m_pool = ctx.enter_context(tc.tile_pool(name="mw", bufs=2))
        m_tiles = {}

        CB = [0, SB - K + 1, 2 * SB - K + 1, 3 * SB - K + 1, L]
        CH = [0, 2 * SB - K + 1, L]

        def mrep(j):
            h, off = (j // 2), CB[j] - CH[j // 2]
            if j % 2 == 0:
                mt_new = m_pool.tile([128, K, 2 * SB], BF16, tag="mw")
                m_tiles[h] = mt_new
            mt = m_tiles[h]
            l0, l1 = CB[j], CB[j + 1]
            nl = l1 - l0
            for k in range(K):
                pmk = p_mk.tile([128, SB], F32, tag="pmk")
                nc.tensor.matmul(
                    pmk[:, :nl],
                    selk_t[:, k, :],
                    en[:, l0 + K - 1 : l0 + K - 1 + nl],
                    start=True,
                    stop=True,
                )
                nc.scalar.copy(mt[:, k, off : off + nl], pmk[:, :nl])

        def conv(c, h, l0, l1, gp3=True):
            nl = l1 - l0
            off = l0 - CH[h]

            def prod(eng, k, pool, tag):
                p = pool.tile([128, 2 * SB], BF16, tag=tag)
                eng.tensor_mul(
                    p[:, :nl],
                    m_tiles[h][:, k, off : off + nl],
                    xtb[:, c, l0 + k : l0 + k + nl],
                )
                return p

            p5 = prod(nc.gpsimd, 5, prodg_pool, "prodg")
            p6 = prod(nc.gpsimd, 6, prodg_pool, "prodg")
            a56 = prodg_pool.tile([128, 2 * SB], BF16, tag="prodg")
            (nc.gpsimd if gp3 else nc.vector).tensor_add(
                a56[:, :nl], p5[:, :nl], p6[:, :nl]
            )
            p0 = prod(nc.vector, 0, prodv_pool, "prodv")
            p1 = prod(nc.vector, 1, prodv_pool, "prodv")
            a01 = prodv_pool.tile([128, 2 * SB], BF16, tag="prodv")
            nc.vector.tensor_add(a01[:, :nl], p0[:, :nl], p1[:, :nl])
            p2 = prod(nc.vector, 2, prodv_pool, "prodv")
            p3 = prod(nc.vector, 3, prodv_pool, "prodv")
            a23 = prodv_pool.tile([128, 2 * SB], BF16, tag="prodv")
            nc.vector.tensor_add(a23[:, :nl], p2[:, :nl], p3[:, :nl])
            p4 = prod(nc.vector, 4, prodv_pool, "prodv")
            t0 = prodv_pool.tile([128, 2 * SB], BF16, tag="prodv")
            nc.vector.tensor_add(t0[:, :nl], a01[:, :nl], a23[:, :nl])
            t1 = prodv_pool.tile([128, 2 * SB], BF16, tag="prodv")
            nc.vector.tensor_add(t1[:, :nl], p4[:, :nl], a56[:, :nl])
            nc.vector.tensor_add(
                acc_all[:, c, l0 : l0 + nl], t0[:, :nl], t1[:, :nl]
            )

        def store(lb):
            l0 = 128 * lb
            nl = min(128, L - l0)
            outn = outn_pool.tile([128, D], F32, tag="outn")
            for half in range(2):
                potp = p_otp.tile([128, 512], BF16, tag="potp")
                for cc in range(4):
                    c = 4 * half + cc
                    nc.tensor.transpose(
                        potp[:nl, 128 * cc : 128 * (cc + 1)],
                        acc_all[:, c, l0 : l0 + nl],
                        identb_t,
                    )
                nc.scalar.copy(outn[:nl, 512 * half : 512 * (half + 1)], potp[:nl, :])
            nc.scalar.dma_start(out=out_d[l0 : l0 + nl, :], in_=outn[:nl, :])

        for j in range(4):
            load(j)
        prev_sums = None
        front(0)
        front(1)
        prev_sums = denom(0)
        mrep(0)
        prev_sums = denom(1)
        mrep(1)
        for c in range(C):
            conv(c, 0, CB[0], CB[1])
        for lb in range(0, 3):
            store(lb)
        front(2, hold=prev_sums)
        prev_sums = denom(2)
        mrep(2)
        for c in range(C):
            conv(c, 0, CB[1], CB[2])
        for lb in range(3, 7):
            store(lb)
        front(3, hold=prev_sums)
        denom(3)
        mrep(3)
        for c in range(C):
            conv(c, 1, CB[2], CB[3])
        for lb in range(7, 11):
            store(lb)
        for c in range(C):
            conv(c, 1, CB[3], CH[2])
        for lb in range(11, 16):
            store(lb)

    nc.compile()
    return nc


_CACHE = {}


def _get_program():
    if "nc" not in _CACHE:
        _CACHE["nc"] = build_program()
    return _CACHE["nc"]


def kernel(x, W, b):
    x = np.asarray(x, dtype=np.float32)
    assert x.shape == (B, S, D), x.shape

    nc = _get_program()
    consts = _host_constants(W, b)
    in_maps = []
    for core in range(B):
        in_maps.append(
            {
                "x": np.ascontiguousarray(x[core]),
                "consts": consts,
            }
        )
    res = bass_utils.run_bass_kernel_spmd(nc, in_maps, core_ids=list(range(B)))
    out = np.stack([res.results[core]["out"] for core in range(B)], axis=0)
    return out
